# revision 1
# baseline (speedup 1.0000x reference)
"""Trainium2 Bass kernel for the SRNN adapter problem.

Strategy (8 cores, data-parallel over batch B=256 -> 32 per core):
  Per scan step t (99 steps), one fused PSUM accumulation per h-tile:
      psum[h] = sum_dk W_inT[dk,h] @ xT[dk, t-cols]   (input projection,
                 no z dependency -> issues while prev step's DVE tail runs)
              + sum_k (W_rec_eff - THR*I)T[k,h] @ z[k] (recurrence)
      v' = alpha*v + psum ; z' = v' > THR ; u' = kappa*u + z'
  Last 10 steps snapshot u; epilogue computes vo_t = W_out @ u_t -> [20, 10*32].
Host: X pre-transposed to [D, T*BL] per core; softmax+mean over the last 10
steps (0.005% of FLOPs).

All matmul arithmetic is fp32: the spiking threshold makes the system chaotic,
so bf16 would decorrelate the spike trains from the fp32 reference. fp32 runs
at 4 cycles/row on the PE, making per-step N=32 matmuls cost the same per
element as wide ones - which is what makes the fused form free.

Toolchain constraint: every engine instruction may carry at most ONE semaphore
wait. Mitigations: state tiles are multi-buffered (WAR deps age out), PSUM is
a single never-released tensor with step-alternating bank sets, and each input
DMA is absorbed by a dummy PE matmul before real consumers run.
"""

import sys

sys.path.insert(0, "/opt/trn_rl_repo")

import numpy as np
from contextlib import ExitStack

from concourse import bacc, bass, mybir, tile
from concourse.bass_utils import run_bass_kernel_spmd

F32 = mybir.dt.float32
A = mybir.AluOpType

B, T, D, H, O = 256, 100, 700, 1024, 20
NCORES = 8
BL = B // NCORES  # 32 batch rows per core
KT = H // 128  # 8 k/h tiles
DTILES = 6  # ceil(700/128), last tile has 60 rows
DLAST = D - 5 * 128  # 60
NSTEPS = T - 1  # 99 scan steps
NTAIL = 10  # last-K softmax window
XCOLS = NSTEPS * BL  # 3168 transposed-X columns actually used

ALPHA = float(np.float32(np.exp(-1.0 / 20.0)))
KAPPA = float(np.float32(np.exp(-1.0 / 20.0)))
THR = 1.0


WI_OFF = 0
WI_LEN = DTILES * H  # 6144
W_OFF = WI_OFF + WI_LEN
W_LEN = KT * H  # 8192
WO_OFF = W_OFF + W_LEN
WO_LEN = KT * O  # 160
XT_OFF = WO_OFF + WO_LEN  # 14496
XT_LEN = DTILES * XCOLS  # 19008
BLOB_COLS = XT_OFF + XT_LEN  # 33504
XT_SPLIT = 256  # xt columns in the early DMA (covers scan steps 0..7)


def _build(nsteps=NSTEPS, nrep=1):
    nc = bacc.Bacc(None)
    blob_d = nc.declare_dram_parameter("blob", [128, BLOB_COLS], F32, isOutput=False)
    vo_d = nc.declare_dram_parameter("vo10", [O, NTAIL * BL], F32, isOutput=True)

    with ExitStack() as ctx:
        tc = ctx.enter_context(tile.TileContext(nc))
        const = ctx.enter_context(tc.tile_pool(name="const", bufs=1))
        pp = ctx.enter_context(tc.tile_pool(name="pp", bufs=1, space="PSUM"))

        blob_sb = const.tile([128, BLOB_COLS], F32, name="blob_sb")
        xt_sb = blob_sb[:, XT_OFF : XT_OFF + XT_LEN].rearrange(
            "p (a c) -> p a c", a=DTILES
        )
        wi_sb = blob_sb[:, WI_OFF : WI_OFF + WI_LEN].rearrange(
            "p (a c) -> p a c", a=DTILES
        )
        w_sb = blob_sb[:, W_OFF : W_OFF + W_LEN].rearrange("p (a c) -> p a c", a=KT)
        wo_sb = blob_sb[:, WO_OFF : WO_OFF + WO_LEN].rearrange(
            "p (a c) -> p a c", a=KT
        )
        v = [const.tile([128, KT, BL], F32, name=f"v{j}") for j in range(3)]
        z = [const.tile([128, KT, BL], F32, name=f"z{j}") for j in range(3)]
        u = [const.tile([128, KT, BL], F32, name=f"u{j}") for j in range(2)]
        usnap = [const.tile([128, KT, BL], F32, name=f"usnap{s}") for s in range(NTAIL)]
        vo_sb = const.tile([O, NTAIL * BL], F32, name="vo_sb")

        # single PSUM tensor, 8 banks: scan step t uses banks (t%2)*4 + h//2,
        # cols (h%2)*BL; vo region [0:20, 0, 64:384]; dummy scratch
        # [0:16, 7, 448:464]
        ps = pp.tile([128, KT, 512], F32, name="ps")

        # split input DMAs: weights first (compute can start ~20us in), then
        # the xt columns for steps 0..31, then the rest streaming under the scan
        xt_dram = blob_d[:, XT_OFF : XT_OFF + XT_LEN].rearrange(
            "p (a c) -> p a c", a=DTILES
        )
        # wi alone first (step 0 needs only wi + early xt); w/wo stream under
        # step 0 and are first needed at step 1
        nc.sync.dma_start(blob_sb[:, 0:W_OFF], blob_d[:, 0:W_OFF])
        nc.sync.dma_start(xt_sb[:, :, 0:XT_SPLIT], xt_dram[:, :, 0:XT_SPLIT])
        nc.sync.dma_start(blob_sb[:, W_OFF:XT_OFF], blob_d[:, W_OFF:XT_OFF])
        nc.sync.dma_start(
            xt_sb[:, :, XT_SPLIT:XCOLS], xt_dram[:, :, XT_SPLIT:XCOLS]
        )

        def dummy_touch(ap):
            # absorb a DMA-completion wait into a cheap PE matmul nobody reads
            return nc.tensor.matmul(
                ps[0:16, 7, 448:464], ap, ap, start=True, stop=True
            )

        dummy_touch(blob_sb[:, 0:16])  # wi
        dummy_touch(xt_sb[:, 0, 0:16])  # early xt

        def step_accum(t, h, with_rec):  # noqa: ANN001
            """Fused accumulation for h-tile h of step t into its psum slot."""
            bank = (t % 2) * 4 + h // 2
            out = ps[:, bank, (h % 2) * BL : (h % 2) * BL + BL]
            n_mm = DTILES + (KT if with_rec else 0)
            i = 0
            for dk in range(DTILES):
                w_ = 128 if dk < 5 else DLAST
                nc.tensor.matmul(
                    out,
                    wi_sb[0:w_, dk, h * 128 : (h + 1) * 128],
                    xt_sb[0:w_, dk, t * BL : (t + 1) * BL],
                    start=(i == 0),
                    stop=(i == n_mm - 1),
                )
                i += 1
            if with_rec:
                for k in range(KT):
                    nc.tensor.matmul(
                        out,
                        w_sb[:, k, h * 128 : (h + 1) * 128],
                        z[t % 3][:, k, :],
                        start=False,
                        stop=(i == n_mm - 1),
                    )
                    i += 1

        for rep in range(nrep):
          for t in range(nsteps):
              cur, nxt = t % 3, (t + 1) % 3
              ucur, unxt = t % 2, (t + 1) % 2
              for h in range(KT):
                  step_accum(t, h, with_rec=(t > 0))
              for p in range(4):
                  hs = slice(2 * p, 2 * p + 2)
                  bank = (t % 2) * 4 + p
                  psum_in = ps[:, bank, 0 : 2 * BL].rearrange("q (a b) -> q a b", a=2)
                  if t == 0:
                      # v = 0*alpha + psum = I_0
                      nc.vector.tensor_copy(v[nxt][:, hs, :], psum_in)
                  else:
                      nc.vector.scalar_tensor_tensor(
                          v[nxt][:, hs, :], v[cur][:, hs, :], ALPHA,
                          psum_in, A.mult, A.add,
                      )
                  nc.vector.tensor_scalar(
                      z[nxt][:, hs, :], v[nxt][:, hs, :], THR, None, A.is_gt
                  )
              if t == 0:
                  nc.vector.tensor_copy(u[unxt][:], z[nxt][:])
              else:
                  nc.vector.scalar_tensor_tensor(
                      u[unxt][:], u[ucur][:], KAPPA, z[nxt][:], A.mult, A.add
                  )
              if nsteps - NTAIL <= t <= nsteps - 1:
                  nc.vector.tensor_copy(usnap[t - (nsteps - NTAIL)][:], u[unxt][:])

        # ---- epilogue: vo = W_out @ u for the last NTAIL steps ----
        vo_ps = ps[0:O, 0, 64 : 64 + NTAIL * BL]
        for s in range(NTAIL):
            for k in range(KT):
                nc.tensor.matmul(
                    vo_ps[:, s * BL : (s + 1) * BL],
                    wo_sb[:, k, :],
                    usnap[s][:, k, :],
                    start=(k == 0),
                    stop=(k == KT - 1),
                )
        nc.vector.tensor_copy(vo_sb[:], vo_ps[:])
        nc.gpsimd.dma_start(vo_d[:], vo_sb[:])

    nc.compile()  # bacc passes legalize multi-wait instructions for TRN2
    return nc


_PROGRAM = None


def _get_program():
    global _PROGRAM
    if _PROGRAM is None:
        _PROGRAM = _build()
    return _PROGRAM


def _host_prep(W_in, W_rec, W_out):
    eye = np.eye(H, dtype=np.float32)
    # z @ w_rec_eff.T - z*THR == z @ (w_rec_eff - THR*eye).T ; lhsT layout [k, h]
    WrT = (W_rec * (1.0 - eye) - np.float32(THR) * eye).T.astype(np.float32)
    WiT = np.zeros((DTILES * 128, H), np.float32)
    WiT[:D] = W_in.T.astype(np.float32)
    WoT = W_out.T.astype(np.float32)  # [H, O]
    # weight section of the blob, identical for every core: [128, cols]
    wpart = np.concatenate(
        [
            WiT.reshape(DTILES, 128, H).transpose(1, 0, 2).reshape(128, -1),
            WrT.reshape(KT, 128, H).transpose(1, 0, 2).reshape(128, -1),
            WoT.reshape(KT, 128, O).transpose(1, 0, 2).reshape(128, -1),
        ],
        axis=1,
    )
    return np.ascontiguousarray(wpart)


def kernel(X, W_in, W_rec, W_out):
    X = np.asarray(X, np.float32)
    wpart = _host_prep(
        np.asarray(W_in, np.float32), np.asarray(W_rec, np.float32),
        np.asarray(W_out, np.float32),
    )
    nc = _get_program()
    in_maps = []
    for c in range(NCORES):
        Xc = X[c * BL : (c + 1) * BL]  # [BL, T, D]
        # [D, t*BL + b] for t = 0..98 (step t uses cols t*BL:(t+1)*BL)
        XTc = np.zeros((DTILES * 128, XCOLS), np.float32)
        XTc[:D] = Xc[:, :NSTEPS, :].transpose(2, 1, 0).reshape(D, XCOLS)
        blob = np.concatenate(
            [wpart,
             XTc.reshape(DTILES, 128, XCOLS).transpose(1, 0, 2).reshape(128, -1)],
            axis=1,
        )
        in_maps.append({"blob": np.ascontiguousarray(blob)})
    res = run_bass_kernel_spmd(nc, in_maps, list(range(NCORES)))
    # vo10 per core: [O, s*BL + b] for scan steps s+89 (vo_full indices 90..99)
    vo = np.stack([r["vo10"] for r in res.results])  # [8, O, 10*BL]
    vo = vo.reshape(NCORES, O, NTAIL, BL).transpose(2, 0, 3, 1).reshape(NTAIL, B, O)
    m = vo.max(axis=2, keepdims=True)
    e = np.exp(vo - m)
    yo = e / e.sum(axis=2, keepdims=True)
    return yo.mean(axis=0).astype(np.float32)

